# revision 1
# baseline (speedup 1.0000x reference)
"""Trainium2 Bass kernel for ConstraintViolationLoss (GNN message passing).

Strategy (8 NeuronCores, SPMD):
  - Host does index-only layout prep: sort edges by constraint, classify each
    constraint into a degree tier (stride 16/24/32/40/48/96...), assign every
    constraint to one of 1024 (core, partition) bins, and pad each
    constraint's edge list to its tier stride so the per-constraint
    segment-sum becomes a fixed-stride reduction.
  - Launch 1 (8 cores, sharded over the small-int vars): softmax
    expected-value head: expected = softmax(logits) @ [0..C) + offsets.
  - Host assembles the per-edge-slot gathered value stream xg (pure indexed
    copies of input values / launch-1 output; no arithmetic).
  - Launch 2 (8 cores, constraints sharded into bins): w = xg * feat,
    per-segment strided reduce -> Ax, violation = relu(Ax - bias), then
    per-partition sum / max / count partials.
  - Host combines 1024 partial rows into the 4 scalar outputs.
"""

import sys

sys.path.insert(0, "/opt/trn_rl_repo")

import numpy as np

import concourse.bass as bass
import concourse.mybir as mybir
from concourse.bass_utils import run_bass_kernel_spmd

P = 128
NCORES = 8
NBINS = P * NCORES
LAMBDA_MEAN, LAMBDA_MAX = 1.0, 0.1
BIAS_COL = 1
LP_SOL_COL = 8
TIER_LS = [16, 24, 32, 40, 48]   # degree-class strides; overflow tier appended
CHUNK_ELEMS = 6144               # target elems/partition per pipelined chunk
BIG_BIAS = 1.0e30
F32 = mybir.dt.float32

# shapes of the most recent build, for test harness introspection
LAST_ROWS_PP = None
LAST_P2_ARGS = None
LA = 48  # legacy alias used by older validation scripts


def _build_phase1(rows_pp: int, nchunks: int = 4, ccls: int = 16):
    """expected = softmax(logits) @ cls + offsets; rows_pp rows/partition."""
    global LAST_ROWS_PP
    LAST_ROWS_PP = rows_pp
    assert rows_pp % nchunks == 0
    rc = rows_pp // nchunks
    fc = rc * ccls
    nf = rows_pp * ccls
    nc = bass.Bass()
    lg = nc.declare_dram_parameter("logits", [P, nf], F32, isOutput=False)
    cp = nc.declare_dram_parameter("clspat", [P, fc], F32, isOutput=False)
    of = nc.declare_dram_parameter("offs", [P, rows_pp], F32, isOutput=False)
    ex = nc.declare_dram_parameter("expected", [P, rows_pp], F32, isOutput=True)

    with (
        nc.sbuf_tensor([P, 2, fc], F32) as tl,
        nc.sbuf_tensor([P, 2, rc], F32) as tof,
        nc.sbuf_tensor([P, fc], F32) as tcp,
        nc.sbuf_tensor([P, fc], F32) as te,
        nc.sbuf_tensor([P, rc], F32) as tden,
        nc.sbuf_tensor([P, rc], F32) as tnum,
        nc.sbuf_tensor([P, rows_pp], F32) as tout,
        nc.Block() as block,
        nc.semaphore("pl0") as pl0,
        nc.semaphore("pl1") as pl1,
        nc.semaphore("csem") as csem,
        nc.semaphore("ssem") as ssem,
        nc.semaphore("vsem") as vsem,
        nc.semaphore("osem") as osem,
    ):
        pl = [pl0, pl1]

        @block.sync
        def _(sync):
            sync.dma_start(out=tcp[:], in_=cp[:]).then_inc(csem, 16)
            for c in range(nchunks):
                if c >= 2:
                    sync.wait_ge(vsem, c - 1)
                b = c % 2
                sync.dma_start(
                    out=tl[:, b, :], in_=lg[:, c * fc : (c + 1) * fc]
                ).then_inc(pl[b], 16)
                sync.dma_start(
                    out=tof[:, b, :], in_=of[:, c * rc : (c + 1) * rc]
                ).then_inc(pl[b], 16)
            sync.wait_ge(vsem, nchunks)
            sync.dma_start(out=ex[:], in_=tout[:]).then_inc(osem, 16)
            sync.wait_ge(osem, 16)

        @block.scalar
        def _(scalar):
            for c in range(nchunks):
                b = c % 2
                scalar.wait_ge(pl[b], 32 * (c // 2 + 1))
                nc.scalar.activation(
                    out=tl[:, b, :], in_=tl[:, b, :],
                    func=mybir.ActivationFunctionType.Exp,
                ).then_inc(ssem, 1)

        @block.vector
        def _(vector):
            vector.wait_ge(csem, 16)
            for c in range(nchunks):
                b = c % 2
                vector.wait_ge(ssem, c + 1)
                nc.vector.drain()
                g = tl[:, b, :].rearrange("p (r c) -> p r c", c=ccls)
                nc.vector.tensor_reduce(
                    out=tden[:], in_=g,
                    axis=mybir.AxisListType.X, op=mybir.AluOpType.add,
                )
                nc.vector.tensor_tensor(
                    out=te[:], in0=tl[:, b, :], in1=tcp[:],
                    op=mybir.AluOpType.mult,
                )
                nc.vector.drain()
                nc.vector.tensor_reduce(
                    out=tnum[:],
                    in_=te[:].rearrange("p (r c) -> p r c", c=ccls),
                    axis=mybir.AxisListType.X, op=mybir.AluOpType.add,
                )
                nc.vector.reciprocal(out=tden[:], in_=tden[:])
                nc.vector.drain()
                nc.vector.tensor_tensor(
                    out=tnum[:], in0=tnum[:], in1=tden[:],
                    op=mybir.AluOpType.mult,
                )
                nc.vector.drain()
                nc.vector.tensor_tensor(
                    out=tout[:, c * rc : (c + 1) * rc],
                    in0=tnum[:], in1=tof[:, b, :], op=mybir.AluOpType.add,
                )
                nc.vector.drain().then_inc(vsem, 1)

    return nc


def _build_phase2(tiers):
    """Per-core segment reduce + loss partials.

    tiers: list of (sa, L, ca) — segments/partition, stride, chunk segments.
    """
    global LAST_P2_ARGS
    LAST_P2_ARGS = (tiers,)
    nc = bass.Bass()
    xg, ft, bs = [], [], []
    for r, (sa, L, ca) in enumerate(tiers):
        xg.append(nc.declare_dram_parameter(f"xg{r}", [P, sa * L], F32, False))
        ft.append(nc.declare_dram_parameter(f"ft{r}", [P, sa * L], F32, False))
        bs.append(nc.declare_dram_parameter(f"bs{r}", [P, sa], F32, False))
    out_p = nc.declare_dram_parameter("partials", [P, 4], F32, isOutput=True)

    fmax = max(ca * L for sa, L, ca in tiers)
    cmax = max(ca for sa, L, ca in tiers)
    chunks = []  # (tier, chunk_idx)
    for r, (sa, L, ca) in enumerate(tiers):
        for i in range(sa // ca):
            chunks.append((r, i))

    with (
        nc.sbuf_tensor([P, 2, fmax], F32) as tx,
        nc.sbuf_tensor([P, 2, fmax], F32) as tf,
        nc.sbuf_tensor([P, 2, cmax], F32) as tb,
        nc.sbuf_tensor([P, cmax], F32) as tax,
        nc.sbuf_tensor([P, cmax], F32) as tviol,
        nc.sbuf_tensor([P, cmax], F32) as tgt,
        nc.sbuf_tensor([P, 1], F32) as ts,
        nc.sbuf_tensor([P, 1], F32) as ts2,
        nc.sbuf_tensor([P, 1], F32) as ts3,
        nc.sbuf_tensor([P, 1], F32) as asum,
        nc.sbuf_tensor([P, 1], F32) as amax,
        nc.sbuf_tensor([P, 1], F32) as acnt,
        nc.sbuf_tensor([P, 4], F32) as tout,
        nc.Block() as block,
        nc.semaphore("pa0") as pa0,
        nc.semaphore("pa1") as pa1,
        nc.semaphore("osem") as osem,
        nc.semaphore("vsem") as vsem,
    ):
        pa = [pa0, pa1]

        @block.sync
        def _(sync):
            for g, (r, i) in enumerate(chunks):
                sa, L, ca = tiers[r]
                fc = ca * L
                if g >= 2:
                    sync.wait_ge(vsem, g - 1)
                b = g % 2
                sync.dma_start(
                    out=tx[:, b, :fc], in_=xg[r][:, i * fc : (i + 1) * fc]
                ).then_inc(pa[b], 16)
                sync.dma_start(
                    out=tf[:, b, :fc], in_=ft[r][:, i * fc : (i + 1) * fc]
                ).then_inc(pa[b], 16)
                sync.dma_start(
                    out=tb[:, b, :ca], in_=bs[r][:, i * ca : (i + 1) * ca]
                ).then_inc(pa[b], 16)
            sync.wait_ge(vsem, len(chunks) + 1)
            sync.dma_start(out=out_p[:], in_=tout[:]).then_inc(osem, 16)
            sync.wait_ge(osem, 16)

        @block.vector
        def _(vector):
            nc.vector.memset(asum[:], 0.0)
            nc.vector.memset(amax[:], 0.0)
            nc.vector.memset(acnt[:], 0.0)

            def seg_chunk(xa, fa_, ba, nseg, ls):
                """Accumulate violation stats for nseg segments of stride ls."""
                nc.vector.drain()
                nc.vector.tensor_tensor(
                    out=xa, in0=xa, in1=fa_, op=mybir.AluOpType.mult
                )
                nc.vector.drain()
                nc.vector.tensor_reduce(
                    out=tax[:, :nseg],
                    in_=xa.rearrange("p (s l) -> p s l", l=ls),
                    axis=mybir.AxisListType.X, op=mybir.AluOpType.add,
                )
                nc.vector.drain()
                nc.vector.tensor_tensor(
                    out=tviol[:, :nseg], in0=tax[:, :nseg], in1=ba,
                    op=mybir.AluOpType.subtract,
                )
                nc.vector.drain()
                nc.vector.tensor_scalar_max(
                    out=tviol[:, :nseg], in0=tviol[:, :nseg], scalar1=0.0
                )
                nc.vector.drain()
                # the three reads of tviol are independent of each other
                nc.vector.tensor_reduce(
                    out=ts[:], in_=tviol[:, :nseg],
                    axis=mybir.AxisListType.X, op=mybir.AluOpType.add,
                )
                nc.vector.tensor_reduce(
                    out=ts2[:], in_=tviol[:, :nseg],
                    axis=mybir.AxisListType.X, op=mybir.AluOpType.max,
                )
                nc.vector.tensor_scalar(
                    out=tgt[:, :nseg], in0=tviol[:, :nseg],
                    scalar1=1e-6, scalar2=None, op0=mybir.AluOpType.is_gt,
                )
                nc.vector.drain()
                nc.vector.tensor_tensor(
                    out=asum[:], in0=asum[:], in1=ts[:], op=mybir.AluOpType.add
                )
                nc.vector.tensor_tensor(
                    out=amax[:], in0=amax[:], in1=ts2[:], op=mybir.AluOpType.max
                )
                nc.vector.tensor_reduce(
                    out=ts3[:], in_=tgt[:, :nseg],
                    axis=mybir.AxisListType.X, op=mybir.AluOpType.add,
                )
                nc.vector.drain()
                nc.vector.tensor_tensor(
                    out=acnt[:], in0=acnt[:], in1=ts3[:], op=mybir.AluOpType.add
                )

            for g, (r, i) in enumerate(chunks):
                sa, L, ca = tiers[r]
                fc = ca * L
                b = g % 2
                vector.wait_ge(pa[b], 48 * (g // 2 + 1))
                seg_chunk(tx[:, b, :fc], tf[:, b, :fc], tb[:, b, :ca], ca, L)
                nc.vector.drain().then_inc(vsem, 1)
            nc.vector.tensor_copy(out=tout[:, 0:1], in_=asum[:])
            nc.vector.tensor_copy(out=tout[:, 1:2], in_=amax[:])
            nc.vector.tensor_copy(out=tout[:, 2:3], in_=acnt[:])
            nc.vector.tensor_copy(out=tout[:, 3:4], in_=acnt[:])
            nc.vector.drain().then_inc(vsem, 1)

    return nc


def _round_up(x: int, m: int) -> int:
    return (x + m - 1) // m * m


def kernel(**inputs) -> tuple:
    prob_bin = np.asarray(inputs["prob_bin"], dtype=np.float32)
    logits = np.asarray(inputs["logits_int_small"], dtype=np.float32)
    offsets = np.asarray(inputs["int_small_offsets"], dtype=np.float32)
    pred_l = np.asarray(inputs["pred_int_large"], dtype=np.float32)
    feat = np.asarray(inputs["edge_features"], dtype=np.float32).reshape(-1)
    cfeat = np.asarray(inputs["constraint_features"], dtype=np.float32)
    vfeat = np.asarray(inputs["variable_features"], dtype=np.float32)
    idx_bin = np.asarray(inputs["idx_bin"], dtype=np.int64)
    idx_s = np.asarray(inputs["idx_int_small"], dtype=np.int64)
    idx_l = np.asarray(inputs["idx_int_large"], dtype=np.int64)
    var_types = np.asarray(inputs["var_types"], dtype=np.int64)
    ei = np.asarray(inputs["edge_indices"], dtype=np.int64)
    n_vars = int(inputs["n_vars"])

    n_con = cfeat.shape[0]
    ns, ccls = logits.shape
    bias = np.ascontiguousarray(cfeat[:, BIAS_COL])
    lp_vals = np.ascontiguousarray(vfeat[:, LP_SOL_COL])
    con = ei[0]
    var = ei[1]
    ne = con.shape[0]

    # ---------------- host index prep (layout only) ----------------
    deg = np.bincount(con, minlength=n_con)
    order = np.argsort(con, kind="stable")
    run_start = np.zeros(n_con + 1, dtype=np.int64)
    np.cumsum(deg, out=run_start[1:])
    off_in_run = np.arange(ne, dtype=np.int64) - run_start[con[order]]
    con_sorted = con[order]
    var_sorted = var[order].astype(np.int32)
    feat_sorted = feat[order]

    max_deg = int(deg.max()) if ne else 0
    strides = list(TIER_LS)
    if max_deg > strides[-1]:
        strides.append(max(96, _round_up(max_deg, 16)))
    # tier id per constraint: first stride >= deg
    tier_of = np.searchsorted(np.asarray(strides), deg, side="left")

    tiers = []        # (sa, L, ca) per tier with any segments
    tier_remap = {}   # original stride index -> dense tier index
    bin_of = np.zeros(n_con, dtype=np.int64)
    rank_of = np.zeros(n_con, dtype=np.int64)
    for si, L in enumerate(strides):
        cons = np.nonzero(tier_of == si)[0]
        if cons.size == 0:
            continue
        rank_order = cons[np.argsort(-deg[cons], kind="stable")]
        ar = np.arange(rank_order.size, dtype=np.int64)
        bin_of[rank_order] = ar % NBINS
        rank_of[rank_order] = ar // NBINS
        sa_need = max(int((rank_order.size + NBINS - 1) // NBINS), 1)
        n_chunks = max(1, -(-sa_need * L // CHUNK_ELEMS))
        ca = -(-sa_need // n_chunks)
        sa = ca * n_chunks
        tier_remap[si] = len(tiers)
        tiers.append((sa, L, ca))

    # per-edge destination slots, per tier
    e_tier = tier_of[con_sorted]
    xgv, ftv, bsv = [], [], []
    for si, r in sorted(tier_remap.items()):
        sa, L, ca = tiers[r]
        sel = e_tier == si
        cs = con_sorted[sel]
        idx = (bin_of[cs] * sa + rank_of[cs]) * L + off_in_run[sel]
        ftr = np.zeros(NBINS * sa * L, dtype=np.float32)
        varr = np.zeros(NBINS * sa * L, dtype=np.int32)
        ftr[idx] = feat_sorted[sel]
        varr[idx] = var_sorted[sel]
        cons = np.nonzero(tier_of == si)[0]
        bsr = np.full(NBINS * sa, BIG_BIAS, dtype=np.float32)
        bsr[bin_of[cons] * sa + rank_of[cons]] = bias[cons]
        ftv.append(ftr)
        xgv.append(varr)
        bsv.append(bsr)

    # ---------------- launch 1: expected values ----------------
    nch1 = 4
    rows_pp = _round_up((ns + NCORES * P - 1) // (NCORES * P), nch1)
    ns_pad = NCORES * P * rows_pp
    lg_pad = np.zeros((ns_pad, ccls), dtype=np.float32)
    lg_pad[:ns] = logits
    of_pad = np.zeros(ns_pad, dtype=np.float32)
    of_pad[:ns] = offsets
    rc = rows_pp // nch1
    clspat = np.tile(np.arange(ccls, dtype=np.float32), rc)[None].repeat(P, 0)

    nc1 = _build_phase1(rows_pp, nch1, ccls)
    lg_r = lg_pad.reshape(NCORES, P, rows_pp * ccls)
    of_r = of_pad.reshape(NCORES, P, rows_pp)
    in1 = [
        {"logits": lg_r[c], "clspat": clspat, "offs": of_r[c]} for c in range(NCORES)
    ]
    res1 = run_bass_kernel_spmd(nc1, in1, list(range(NCORES)))
    expected = np.concatenate(
        [res1.results[c]["expected"].reshape(-1) for c in range(NCORES)]
    )[:ns]

    # ---------------- host: assemble x and gather streams ----------------
    xfull = np.zeros(n_vars, dtype=np.float32)
    xfull[idx_bin] = prob_bin[:, 0]
    xfull[idx_s] = expected
    xfull[idx_l] = pred_l[:, 0]
    xfull = np.where(var_types == 0, lp_vals, xfull)

    # ---------------- launch 2: segment reduce + loss partials ----------------
    nc2 = _build_phase2(tiers)
    in2 = []
    for c in range(NCORES):
        m = {}
        for r, (sa, L, ca) in enumerate(tiers):
            m[f"xg{r}"] = xfull[xgv[r].reshape(NCORES, P, sa * L)[c]]
            m[f"ft{r}"] = ftv[r].reshape(NCORES, P, sa * L)[c]
            m[f"bs{r}"] = bsv[r].reshape(NCORES, P, sa)[c]
        in2.append(m)
    res2 = run_bass_kernel_spmd(nc2, in2, list(range(NCORES)))

    parts = np.stack([res2.results[c]["partials"] for c in range(NCORES)])
    vsum = np.float32(parts[:, :, 0].astype(np.float64).sum())
    vmax = np.float32(parts[:, :, 1].max())
    vcnt = np.int32(round(float(parts[:, :, 2].sum())))
    mean_viol = np.float32(vsum / np.float32(n_con))
    penalty = np.float32(
        np.float32(LAMBDA_MEAN) * mean_viol + np.float32(LAMBDA_MAX) * vmax
    )
    return penalty, mean_viol, vmax, vcnt



# revision 19
# speedup vs baseline: 1.6264x; 1.6264x over previous
"""Trainium2 Bass kernel for ConstraintViolationLoss (GNN message passing).

Strategy (8 NeuronCores, SPMD, bf16 streams):
  - Host does index-only layout prep: sort edges by constraint, pad each
    constraint's edge list to a degree-tier stride L (granularity 4) plus one
    extra slot carrying (1, -bias), so the per-constraint segment-sum directly
    yields Ax - bias with no separate bias stream or subtract.
  - Launch 1 (softmax expected-value head): exp on the scalar engine; the
    class-weighted numerator and denominator reduced together as one bf16
    pairwise add-tree (4D view) on the vector engine; reciprocal + combine.
  - Host assembles the per-edge-slot gathered value stream (pure indexed
    copies of input values / launch-1 output; no arithmetic).
  - Launch 2: DMA chunks (all degree tiers coalesced, small first/last chunk
    to shorten pipeline ends, 4-deep ring buffer). Per chunk the vector
    engine does one flat bf16 multiply and per-tier in-place pairwise
    add-tree segment sums; the Pool engine applies relu + threshold compare;
    final sum/max/count reductions once at the end.
  - Host combines the 1024 per-partition partial rows into the 4 scalars.
"""

import sys

sys.path.insert(0, "/opt/trn_rl_repo")

import numpy as np

import concourse.bass as bass
import concourse.mybir as mybir
from concourse.bass_utils import run_bass_kernel_spmd

P = 128
NCORES = 8
NBINS = P * NCORES
LAMBDA_MEAN, LAMBDA_MAX = 1.0, 0.1
BIAS_COL = 1
LP_SOL_COL = 8
CHUNK_SLOTS = 3584        # stream slots / partition in the middle chunks
EDGE_SLOTS = 1024         # first and last chunk size (pipeline fill/drain)
NRING = 4                 # phase-2 ring buffers
F32 = mybir.dt.float32
BF16 = mybir.dt.bfloat16
NP_BF16 = mybir.dt.np(BF16)

# shapes of the most recent build, for test harness introspection
LAST_P1_ARGS = None
LAST_P2_ARGS = None


def _tree_reduce(nc, xv, out, width):
    """Sum the last axis of xv [..., width] into out [...].

    In-place pairwise halvings (bf16 tensor_tensor add, 2x DVE mode) while the
    width stays even, then one tensor_reduce for the remainder. Returns the
    final emitted instruction (the last reader of xv's buffer).
    """
    h = width
    while h % 2 == 0 and h > 2:
        nh = h // 2
        nc.vector.drain()
        nc.vector.tensor_tensor(
            out=xv[..., :nh], in0=xv[..., :nh], in1=xv[..., nh:h],
            op=mybir.AluOpType.add,
        )
        h = nh
    nc.vector.drain()
    with nc.allow_low_precision("bf16 partial sums, validated vs f64 oracle"):
        return nc.vector.tensor_reduce(
            out=out, in_=xv[..., :h],
            axis=mybir.AxisListType.X, op=mybir.AluOpType.add,
        )


def _build_phase1(rcs, ccls=16):
    """expected = softmax(logits) @ cls + offsets.

    rcs: rows/partition per compute chunk (first small for pipeline fill).
    Stream layout: [logits chunk 0 | ... | logits chunk n-1 | offsets(all)].
    """
    rows_pp = sum(rcs)
    rcmax = max(rcs)
    nch = len(rcs)
    lgmax = rcmax * ccls
    nc = bass.Bass()
    st = nc.declare_dram_parameter(
        "stream", [P, rows_pp * (ccls + 1)], BF16, isOutput=False
    )
    cp = nc.declare_dram_parameter("clspat", [P, lgmax], BF16, isOutput=False)
    ex = nc.declare_dram_parameter("expected", [P, rows_pp], BF16, isOutput=True)

    lgoff = [0]
    for rc in rcs:
        lgoff.append(lgoff[-1] + rc * ccls)
    roff = [0]
    for rc in rcs:
        roff.append(roff[-1] + rc)

    with (
        nc.sbuf_tensor([P, lgoff[-1] * 2], BF16) as wt,
        nc.sbuf_tensor([P, rows_pp], BF16) as toff,
        nc.sbuf_tensor([P, lgmax], BF16) as tcp,
        nc.sbuf_tensor([P, 2, rcmax], F32) as dn,
        nc.sbuf_tensor([P, rcmax], F32) as rden,
        nc.sbuf_tensor([P, rcmax], F32) as tmu,
        nc.sbuf_tensor([P, rows_pp], BF16) as tout,
        nc.Block() as block,
        nc.semaphore("cs1") as cs1,
        nc.semaphore("cs2") as cs2,
        nc.semaphore("cs3") as cs3,
        nc.semaphore("asem") as asem,
        nc.semaphore("vsem") as vsem,
        nc.semaphore("osem") as osem,
    ):
        dsems = [nc.alloc_semaphore(f"dsem{g}") for g in range(nch)]

        @block.sync
        def _(sync):
            sync.dma_start(
                out=wt[:, 2 * lgoff[0] : 2 * lgoff[0] + rcs[0] * ccls],
                in_=st[:, lgoff[0] : lgoff[1]],
            ).then_inc(dsems[0], 16)
            sync.dma_start(
                out=tcp[:, : rcs[0] * ccls], in_=cp[:, : rcs[0] * ccls]
            ).then_inc(cs1, 16)
            sync.dma_start(
                out=toff[:], in_=st[:, lgoff[-1] :]
            ).then_inc(cs3, 16)
            sync.dma_start(
                out=tcp[:, rcs[0] * ccls :], in_=cp[:, rcs[0] * ccls :]
            ).then_inc(cs2, 16)
            for g in range(1, nch):
                sync.dma_start(
                    out=wt[:, 2 * lgoff[g] : 2 * lgoff[g] + rcs[g] * ccls],
                    in_=st[:, lgoff[g] : lgoff[g + 1]],
                ).then_inc(dsems[g], 16)
            for g in range(nch):
                sync.wait_ge(vsem, g + 1)
                sync.dma_start(
                    out=ex[:, roff[g] : roff[g + 1]],
                    in_=tout[:, roff[g] : roff[g + 1]],
                ).then_inc(osem, 16)
            sync.wait_ge(osem, 16 * nch)

        @block.scalar
        def _(scalar):
            for g in range(nch):
                lg = wt[:, 2 * lgoff[g] : 2 * lgoff[g] + rcs[g] * ccls]
                scalar.wait_ge(dsems[g], 16)
                nc.scalar.activation(
                    out=lg, in_=lg, func=mybir.ActivationFunctionType.Exp,
                ).then_inc(asem, 1)

        @block.vector
        def _(vector):
            vector.wait_ge(cs1, 16)
            vector.wait_ge(cs3, 16)
            for g, rc in enumerate(rcs):
                lgn = rc * ccls
                w2 = wt[:, 2 * lgoff[g] : 2 * lgoff[g] + 2 * lgn]
                if g == 1:
                    vector.wait_ge(cs2, 16)
                vector.wait_ge(asem, g + 1)
                # numerator products next to exp so one 4D tree reduces both
                nc.vector.tensor_tensor(
                    out=w2[:, lgn:], in0=w2[:, :lgn], in1=tcp[:, :lgn],
                    op=mybir.AluOpType.mult,
                )
                v4 = w2.rearrange("p (t r c) -> p t r c", t=2, c=ccls)
                _tree_reduce(nc, v4, dn[:, :, :rc], ccls)
                nc.vector.drain()
                nc.vector.reciprocal(out=rden[:, :rc], in_=dn[:, 0, :rc])
                nc.vector.drain()
                nc.vector.tensor_tensor(
                    out=tmu[:, :rc], in0=dn[:, 1, :rc], in1=rden[:, :rc],
                    op=mybir.AluOpType.mult,
                )
                nc.vector.drain()
                nc.vector.tensor_tensor(
                    out=tout[:, roff[g] : roff[g + 1]],
                    in0=tmu[:, :rc], in1=toff[:, roff[g] : roff[g + 1]],
                    op=mybir.AluOpType.add,
                )
                nc.vector.drain().then_inc(vsem, 1)

    return nc


def _build_phase2(chunks, spp):
    """Per-core segment reduce + loss partials, single bf16 stream.

    chunks: list of (soff, fc, subblocks); subblocks: (sub_off, ca, Lp, vpos).
    Chunk g stream block is [xg subs... | ft subs...] at soff, 2*fc wide.
    spp: total segments per partition.
    """
    nch = len(chunks)
    fcmax = max(c[1] for c in chunks)
    ftot = chunks[-1][0] + 2 * chunks[-1][1]
    nc = bass.Bass()
    st = nc.declare_dram_parameter("stream", [P, ftot], BF16, isOutput=False)
    out_p = nc.declare_dram_parameter("partials", [P, 3], F32, isOutput=True)

    with (
        nc.sbuf_tensor([P, NRING, 2 * fcmax], BF16) as ts,
        nc.sbuf_tensor([P, spp], BF16) as viol,
        nc.sbuf_tensor([P, spp], BF16) as gtv,
        nc.sbuf_tensor([P, 3], F32) as tout,
        nc.Block() as block,
        nc.semaphore("vsem") as vsem,
        nc.semaphore("psem") as psem,
        nc.semaphore("esem") as esem,
        nc.semaphore("osem") as osem,
    ):
        dsems = [nc.alloc_semaphore(f"dsem{b}") for b in range(NRING)]

        @block.sync
        def _(sync):
            for g, (soff, fc, subs) in enumerate(chunks):
                if g >= NRING:
                    sync.wait_ge(vsem, g - NRING + 1)
                b = g % NRING
                sync.dma_start(
                    out=ts[:, b, : 2 * fc], in_=st[:, soff : soff + 2 * fc]
                ).then_inc(dsems[b], 16)
            sync.wait_ge(esem, 1)
            sync.dma_start(out=out_p[:], in_=tout[:]).then_inc(osem, 16)
            sync.wait_ge(osem, 16)

        @block.vector
        def _(vector):
            for g, (soff, fc, subs) in enumerate(chunks):
                b = g % NRING
                vector.wait_ge(dsems[b], 16 * (g // NRING + 1))
                nc.vector.tensor_tensor(
                    out=ts[:, b, :fc], in0=ts[:, b, :fc],
                    in1=ts[:, b, fc : 2 * fc], op=mybir.AluOpType.mult,
                )
                for (sub_off, ca, Lp, vpos) in subs:
                    xv = ts[:, b, sub_off : sub_off + ca * Lp].rearrange(
                        "p (s l) -> p s l", l=Lp
                    )
                    _tree_reduce(nc, xv, viol[:, vpos : vpos + ca], Lp)
                nc.vector.drain().then_inc(vsem, 1)
            # end tail: stats over all segments (after Pool relu/compare)
            vector.wait_ge(psem, nch)
            nc.vector.tensor_reduce(
                out=tout[:, 0:1], in_=viol[:],
                axis=mybir.AxisListType.X, op=mybir.AluOpType.add,
            )
            nc.vector.tensor_reduce(
                out=tout[:, 1:2], in_=viol[:],
                axis=mybir.AxisListType.X, op=mybir.AluOpType.max,
            )
            nc.vector.tensor_reduce(
                out=tout[:, 2:3], in_=gtv[:],
                axis=mybir.AxisListType.X, op=mybir.AluOpType.add,
            )
            nc.vector.drain().then_inc(esem, 1)

        @block.gpsimd
        def _(gpsimd):
            for g, (soff, fc, subs) in enumerate(chunks):
                v0 = subs[0][3]
                v1 = subs[-1][3] + subs[-1][1]
                gpsimd.wait_ge(vsem, g + 1)
                vs = viol[:, v0:v1]
                nc.gpsimd.tensor_scalar_max(out=vs, in0=vs, scalar1=0.0)
                nc.gpsimd.drain()
                nc.gpsimd.tensor_scalar(
                    out=gtv[:, v0:v1], in0=vs,
                    scalar1=1e-6, scalar2=None, op0=mybir.AluOpType.is_gt,
                )
                nc.gpsimd.drain().then_inc(psem, 1)

    return nc


def _stride_of(deg):
    """Allowed segment strides (edge slots + 1 bias slot) per degree."""
    need = deg + 1
    lp = np.maximum(24, ((need + 3) // 4) * 4)       # 24,28,...,44
    lp = np.where(need > 44, ((need + 7) // 8) * 8, lp)   # 48,56
    lp = np.where(need > 56, ((need + 15) // 16) * 16, lp)  # 64,80,...
    return lp.astype(np.int64)


def _plan_chunks(tier_list):
    """Pack (Lp, sa) tiers into DMA chunks of subblocks.

    First and last chunks are ~EDGE_SLOTS to shorten pipeline fill/drain;
    middles ~CHUNK_SLOTS. Returns (chunks, spp, tier_map): tier_map[i] is a
    list of (rank_start, rank_end, chunk_soff, sub_off, fc_of_chunk) spans.
    """
    total = sum(Lp * sa for (Lp, sa) in tier_list)
    mid_n = max(1, -(-(total - 2 * EDGE_SLOTS) // CHUNK_SLOTS))
    mid = (total - 2 * EDGE_SLOTS) // mid_n
    budgets = [EDGE_SLOTS] + [mid] * mid_n + [2 * EDGE_SLOTS]

    # ascending stride: small-tier subblock spam lands in the early chunks
    # where the vector engine still has slack
    order = sorted(range(len(tier_list)), key=lambda i: tier_list[i][0])
    raw_chunks = [[]]
    fill = 0
    for i in order:
        Lp, sa = tier_list[i]
        r = 0
        while r < sa:
            budget = budgets[min(len(raw_chunks) - 1, len(budgets) - 1)]
            room = (budget - fill) // Lp
            if room == 0:
                raw_chunks.append([])
                fill = 0
                continue
            take = min(sa - r, room)
            raw_chunks[-1].append((i, r, take, Lp))
            fill += take * Lp
            r += take

    chunks = []
    tier_map = {i: [] for i in range(len(tier_list))}
    soff = vpos = 0
    for ch in raw_chunks:
        fc = sum(t * lp for (_, _, t, lp) in ch)
        subs = []
        sub_off = 0
        for (i, r, take, Lp) in ch:
            subs.append((sub_off, take, Lp, vpos))
            tier_map[i].append((r, r + take, soff, sub_off, fc))
            sub_off += take * Lp
            vpos += take
        chunks.append((soff, fc, subs))
        soff += 2 * fc
    return chunks, vpos, tier_map


def kernel(**inputs) -> tuple:
    global LAST_P1_ARGS, LAST_P2_ARGS
    prob_bin = np.asarray(inputs["prob_bin"], dtype=np.float32)
    logits = np.asarray(inputs["logits_int_small"], dtype=np.float32)
    offsets = np.asarray(inputs["int_small_offsets"], dtype=np.float32)
    pred_l = np.asarray(inputs["pred_int_large"], dtype=np.float32)
    feat = np.asarray(inputs["edge_features"], dtype=np.float32).reshape(-1)
    cfeat = np.asarray(inputs["constraint_features"], dtype=np.float32)
    vfeat = np.asarray(inputs["variable_features"], dtype=np.float32)
    idx_bin = np.asarray(inputs["idx_bin"], dtype=np.int64)
    idx_s = np.asarray(inputs["idx_int_small"], dtype=np.int64)
    idx_l = np.asarray(inputs["idx_int_large"], dtype=np.int64)
    var_types = np.asarray(inputs["var_types"], dtype=np.int64)
    ei = np.asarray(inputs["edge_indices"], dtype=np.int64)
    n_vars = int(inputs["n_vars"])

    n_con = cfeat.shape[0]
    ns, ccls = logits.shape
    bias = np.ascontiguousarray(cfeat[:, BIAS_COL])
    lp_vals = np.ascontiguousarray(vfeat[:, LP_SOL_COL])
    con = ei[0]
    var = ei[1]
    ne = con.shape[0]

    # ---------------- launch 1: expected values ----------------
    rows_pp = -(-ns // NBINS)
    rcs = [16, 48]
    rmid = -(-(rows_pp - 64) // 3)
    while sum(rcs) < rows_pp:
        rcs.append(min(rmid, rows_pp - sum(rcs)))
    rcmax = max(rcs)
    ns_pad = NBINS * rows_pp
    lg_pad = np.zeros((ns_pad, ccls), dtype=NP_BF16)
    lg_pad[:ns] = logits
    of_pad = np.zeros(ns_pad, dtype=NP_BF16)
    of_pad[:ns] = offsets
    lg_r = lg_pad.reshape(NCORES, P, rows_pp * ccls)
    of_r = of_pad.reshape(NCORES, P, rows_pp)
    stream1 = np.concatenate([lg_r, of_r], axis=2)
    clspat = np.ascontiguousarray(
        np.tile(np.arange(ccls, dtype=NP_BF16), rcmax)[None].repeat(P, 0)
    )

    LAST_P1_ARGS = (rcs, ccls)
    nc1 = _build_phase1(*LAST_P1_ARGS)
    in1 = [{"stream": stream1[c], "clspat": clspat} for c in range(NCORES)]
    res1 = run_bass_kernel_spmd(nc1, in1, list(range(NCORES)))
    expected = np.concatenate(
        [res1.results[c]["expected"].reshape(-1) for c in range(NCORES)]
    )[:ns].astype(np.float32)

    # ---------------- host: assemble x (indexed copies) ----------------
    xfull = np.zeros(n_vars, dtype=np.float32)
    xfull[idx_bin] = prob_bin[:, 0]
    xfull[idx_s] = expected
    xfull[idx_l] = pred_l[:, 0]
    xfull = np.where(var_types == 0, lp_vals, xfull)
    xfull_bf = xfull.astype(NP_BF16)
    feat_bf = feat.astype(NP_BF16)
    nbias_bf = (-bias).astype(NP_BF16)

    # ---------------- host index prep (layout only) ----------------
    deg = np.bincount(con, minlength=n_con)
    order = np.argsort(con, kind="stable")
    run_start = np.zeros(n_con + 1, dtype=np.int64)
    np.cumsum(deg, out=run_start[1:])
    off_in_run = np.arange(ne, dtype=np.int64) - run_start[con[order]]
    con_sorted = con[order]
    var_sorted = var[order]
    feat_sorted = feat_bf[order]

    # stride tier per constraint: smallest allowed stride >= deg+1 (bias
    # slot). Few tiers keep the per-subblock instruction count low; strides
    # mostly multiples of 4 where the degree mass is.
    Lp_of = _stride_of(deg)
    tier_Ls = np.unique(Lp_of)

    # per tier: bin/rank assignment
    bin_of = np.zeros(n_con, dtype=np.int64)
    rank_of = np.zeros(n_con, dtype=np.int64)
    tier_list = []
    for Lp in tier_Ls:
        cons = np.nonzero(Lp_of == Lp)[0]
        rank_order = cons[np.argsort(-deg[cons], kind="stable")]
        ar = np.arange(rank_order.size, dtype=np.int64)
        bin_of[rank_order] = ar % NBINS
        rank_of[rank_order] = ar // NBINS
        sa = max(int(-(-rank_order.size // NBINS)), 1)
        tier_list.append((int(Lp), sa))

    chunks, spp, tier_map = _plan_chunks(tier_list)
    ftot = chunks[-1][0] + 2 * chunks[-1][1]

    # ---------------- host: scatter the edge stream ----------------
    stream2 = np.zeros(NBINS * ftot, dtype=NP_BF16)
    e_Lp = Lp_of[con_sorted]
    for t, (Lp, sa) in enumerate(tier_list):
        spans = tier_map[t]
        rstarts = np.array([s[0] for s in spans], dtype=np.int64)
        base = np.array(
            [soff + sub_off for (_, _, soff, sub_off, _) in spans],
            dtype=np.int64,
        )
        fcs = np.array([s[4] for s in spans], dtype=np.int64)

        def locs(ranks, slot):
            si = np.searchsorted(rstarts, ranks, side="right") - 1
            loc = base[si] + (ranks - rstarts[si]) * Lp + slot
            return loc, fcs[si]

        sel = np.nonzero(e_Lp == Lp)[0]
        cs = con_sorted[sel]
        loc, fc_e = locs(rank_of[cs], off_in_run[sel])
        flat = bin_of[cs] * ftot + loc
        stream2[flat] = xfull_bf[var_sorted[sel]]
        stream2[flat + fc_e] = feat_sorted[sel]
        # bias slot per real segment: (1, -bias) at slot Lp-1
        cons = np.nonzero(Lp_of == Lp)[0]
        locb, fc_b = locs(rank_of[cons], Lp - 1)
        flatb = bin_of[cons] * ftot + locb
        stream2[flatb] = np.array(1.0, dtype=NP_BF16)
        stream2[flatb + fc_b] = nbias_bf[cons]
    stream2 = stream2.reshape(NCORES, P, ftot)

    # ---------------- launch 2: segment reduce + loss partials ----------------
    LAST_P2_ARGS = (chunks, spp)
    nc2 = _build_phase2(*LAST_P2_ARGS)
    in2 = [{"stream": stream2[c]} for c in range(NCORES)]
    res2 = run_bass_kernel_spmd(nc2, in2, list(range(NCORES)))

    parts = np.stack([res2.results[c]["partials"] for c in range(NCORES)])
    vsum = parts[:, :, 0].astype(np.float64).sum()
    vmax = np.float32(max(parts[:, :, 1].max(), 0.0))
    vcnt = np.int32(round(float(parts[:, :, 2].astype(np.float64).sum())))
    mean_viol = np.float32(vsum / n_con)
    penalty = np.float32(
        np.float32(LAMBDA_MEAN) * mean_viol + np.float32(LAMBDA_MAX) * vmax
    )
    return penalty, mean_viol, vmax, vcnt


# revision 22
# speedup vs baseline: 1.6452x; 1.0116x over previous
"""Trainium2 Bass kernel for ConstraintViolationLoss (GNN message passing).

Strategy (8 NeuronCores, SPMD, bf16 streams):
  - Host does index-only layout prep: sort edges by constraint, pad each
    constraint's edge list to a degree-tier stride L (granularity 4) plus one
    extra slot carrying (1, -bias), so the per-constraint segment-sum directly
    yields Ax - bias with no separate bias stream or subtract.
  - Launch 1 (softmax expected-value head): exp on the scalar engine; the
    class-weighted numerator and denominator reduced together as one bf16
    pairwise add-tree (4D view) on the vector engine; reciprocal + combine.
  - Host assembles the per-edge-slot gathered value stream (pure indexed
    copies of input values / launch-1 output; no arithmetic).
  - Launch 2: DMA chunks (all degree tiers coalesced, small first/last chunk
    to shorten pipeline ends, 4-deep ring buffer). Per chunk the vector
    engine does one flat bf16 multiply and per-tier in-place pairwise
    add-tree segment sums; the Pool engine applies relu + threshold compare;
    final sum/max/count reductions once at the end.
  - Host combines the 1024 per-partition partial rows into the 4 scalars.
"""

import sys

sys.path.insert(0, "/opt/trn_rl_repo")

import numpy as np

import concourse.bass as bass
import concourse.mybir as mybir
from concourse.bass_utils import run_bass_kernel_spmd

P = 128
NCORES = 8
NBINS = P * NCORES
LAMBDA_MEAN, LAMBDA_MAX = 1.0, 0.1
BIAS_COL = 1
LP_SOL_COL = 8
CHUNK_SLOTS = 3584        # stream slots / partition in the middle chunks
EDGE_SLOTS = 1024         # first and last chunk size (pipeline fill/drain)
NRING = 4                 # phase-2 ring buffers
POOL_FRAC = 0.27          # fraction of each chunk's multiply done on Pool
F32 = mybir.dt.float32
BF16 = mybir.dt.bfloat16
NP_BF16 = mybir.dt.np(BF16)

# shapes of the most recent build, for test harness introspection
LAST_P1_ARGS = None
LAST_P2_ARGS = None


def _tree_reduce(nc, xv, out, width):
    """Sum the last axis of xv [..., width] into out [...].

    In-place pairwise halvings (bf16 tensor_tensor add, 2x DVE mode) while the
    width stays even, then one tensor_reduce for the remainder. Returns the
    final emitted instruction (the last reader of xv's buffer).
    """
    h = width
    while h % 2 == 0 and h > 2:
        nh = h // 2
        nc.vector.drain()
        nc.vector.tensor_tensor(
            out=xv[..., :nh], in0=xv[..., :nh], in1=xv[..., nh:h],
            op=mybir.AluOpType.add,
        )
        h = nh
    nc.vector.drain()
    with nc.allow_low_precision("bf16 partial sums, validated vs f64 oracle"):
        return nc.vector.tensor_reduce(
            out=out, in_=xv[..., :h],
            axis=mybir.AxisListType.X, op=mybir.AluOpType.add,
        )


def _build_phase1(rcs, ccls=16):
    """expected = softmax(logits) @ cls + offsets.

    rcs: rows/partition per compute chunk (first small for pipeline fill).
    Stream layout: [logits chunk 0 | ... | logits chunk n-1 | offsets(all)].
    """
    rows_pp = sum(rcs)
    rcmax = max(rcs)
    nch = len(rcs)
    lgmax = rcmax * ccls
    nc = bass.Bass()
    st = nc.declare_dram_parameter(
        "stream", [P, rows_pp * (ccls + 1)], BF16, isOutput=False
    )
    cp = nc.declare_dram_parameter("clspat", [P, lgmax], BF16, isOutput=False)
    ex = nc.declare_dram_parameter("expected", [P, rows_pp], BF16, isOutput=True)

    lgoff = [0]
    for rc in rcs:
        lgoff.append(lgoff[-1] + rc * ccls)
    roff = [0]
    for rc in rcs:
        roff.append(roff[-1] + rc)

    with (
        nc.sbuf_tensor([P, lgoff[-1] * 2], BF16) as wt,
        nc.sbuf_tensor([P, rows_pp], BF16) as toff,
        nc.sbuf_tensor([P, lgmax], BF16) as tcp,
        nc.sbuf_tensor([P, 2, rcmax], F32) as dn,
        nc.sbuf_tensor([P, rcmax], F32) as rden,
        nc.sbuf_tensor([P, rcmax], F32) as tmu,
        nc.sbuf_tensor([P, rows_pp], BF16) as tout,
        nc.Block() as block,
        nc.semaphore("cs1") as cs1,
        nc.semaphore("cs2") as cs2,
        nc.semaphore("cs3") as cs3,
        nc.semaphore("asem") as asem,
        nc.semaphore("vsem") as vsem,
        nc.semaphore("osem") as osem,
    ):
        dsems = [nc.alloc_semaphore(f"dsem{g}") for g in range(nch)]

        @block.sync
        def _(sync):
            sync.dma_start(
                out=wt[:, 2 * lgoff[0] : 2 * lgoff[0] + rcs[0] * ccls],
                in_=st[:, lgoff[0] : lgoff[1]],
            ).then_inc(dsems[0], 16)
            sync.dma_start(
                out=tcp[:, : rcs[0] * ccls], in_=cp[:, : rcs[0] * ccls]
            ).then_inc(cs1, 16)
            sync.dma_start(
                out=toff[:], in_=st[:, lgoff[-1] :]
            ).then_inc(cs3, 16)
            sync.dma_start(
                out=tcp[:, rcs[0] * ccls :], in_=cp[:, rcs[0] * ccls :]
            ).then_inc(cs2, 16)
            for g in range(1, nch):
                sync.dma_start(
                    out=wt[:, 2 * lgoff[g] : 2 * lgoff[g] + rcs[g] * ccls],
                    in_=st[:, lgoff[g] : lgoff[g + 1]],
                ).then_inc(dsems[g], 16)
            for g in range(nch):
                sync.wait_ge(vsem, g + 1)
                sync.dma_start(
                    out=ex[:, roff[g] : roff[g + 1]],
                    in_=tout[:, roff[g] : roff[g + 1]],
                ).then_inc(osem, 16)
            sync.wait_ge(osem, 16 * nch)

        @block.scalar
        def _(scalar):
            for g in range(nch):
                lg = wt[:, 2 * lgoff[g] : 2 * lgoff[g] + rcs[g] * ccls]
                scalar.wait_ge(dsems[g], 16)
                nc.scalar.activation(
                    out=lg, in_=lg, func=mybir.ActivationFunctionType.Exp,
                ).then_inc(asem, 1)

        @block.vector
        def _(vector):
            vector.wait_ge(cs1, 16)
            vector.wait_ge(cs3, 16)
            for g, rc in enumerate(rcs):
                lgn = rc * ccls
                w2 = wt[:, 2 * lgoff[g] : 2 * lgoff[g] + 2 * lgn]
                if g == 1:
                    vector.wait_ge(cs2, 16)
                vector.wait_ge(asem, g + 1)
                # numerator products next to exp so one 4D tree reduces both
                nc.vector.tensor_tensor(
                    out=w2[:, lgn:], in0=w2[:, :lgn], in1=tcp[:, :lgn],
                    op=mybir.AluOpType.mult,
                )
                v4 = w2.rearrange("p (t r c) -> p t r c", t=2, c=ccls)
                _tree_reduce(nc, v4, dn[:, :, :rc], ccls)
                nc.vector.drain()
                nc.vector.reciprocal(out=rden[:, :rc], in_=dn[:, 0, :rc])
                nc.vector.drain()
                nc.vector.tensor_tensor(
                    out=tmu[:, :rc], in0=dn[:, 1, :rc], in1=rden[:, :rc],
                    op=mybir.AluOpType.mult,
                )
                nc.vector.drain()
                nc.vector.tensor_tensor(
                    out=tout[:, roff[g] : roff[g + 1]],
                    in0=tmu[:, :rc], in1=toff[:, roff[g] : roff[g + 1]],
                    op=mybir.AluOpType.add,
                )
                nc.vector.drain().then_inc(vsem, 1)

    return nc


def _build_phase2(chunks, spp):
    """Per-core segment reduce + loss partials, single bf16 stream.

    chunks: list of (soff, fc, subblocks); subblocks: (sub_off, ca, Lp, vpos).
    Chunk g stream block is [xg subs... | ft subs...] at soff, 2*fc wide.
    spp: total segments per partition.
    """
    nch = len(chunks)
    fcmax = max(c[1] for c in chunks)
    ftot = chunks[-1][0] + 2 * chunks[-1][1]
    nc = bass.Bass()
    st = nc.declare_dram_parameter("stream", [P, ftot], BF16, isOutput=False)
    out_p = nc.declare_dram_parameter("partials", [P, 3], F32, isOutput=True)

    with (
        nc.sbuf_tensor([P, NRING, 2 * fcmax], BF16) as ts,
        nc.sbuf_tensor([P, spp], BF16) as viol,
        nc.sbuf_tensor([P, spp], BF16) as gtv,
        nc.sbuf_tensor([P, 3], F32) as tout,
        nc.Block() as block,
        nc.semaphore("vsem") as vsem,
        nc.semaphore("psem") as psem,
        nc.semaphore("esem") as esem,
        nc.semaphore("osem") as osem,
    ):
        dsems = [nc.alloc_semaphore(f"dsem{b}") for b in range(NRING)]
        pmsem = nc.alloc_semaphore("pmsem")
        # Pool multiplies the leading POOL_FRAC of each chunk's slots
        pms = [int(fc * POOL_FRAC) for (_, fc, _) in chunks]

        @block.sync
        def _(sync):
            for g, (soff, fc, subs) in enumerate(chunks):
                if g >= NRING:
                    sync.wait_ge(vsem, g - NRING + 1)
                b = g % NRING
                sync.dma_start(
                    out=ts[:, b, : 2 * fc], in_=st[:, soff : soff + 2 * fc]
                ).then_inc(dsems[b], 16)
            sync.wait_ge(esem, 1)
            sync.dma_start(out=out_p[:], in_=tout[:]).then_inc(osem, 16)
            sync.wait_ge(osem, 16)

        @block.vector
        def _(vector):
            for g, (soff, fc, subs) in enumerate(chunks):
                b = g % NRING
                vector.wait_ge(dsems[b], 16 * (g // NRING + 1))
                pm = pms[g]
                nc.vector.tensor_tensor(
                    out=ts[:, b, pm:fc], in0=ts[:, b, pm:fc],
                    in1=ts[:, b, fc + pm : 2 * fc], op=mybir.AluOpType.mult,
                )
                vector.wait_ge(pmsem, g + 1)
                for (sub_off, ca, Lp, vpos) in subs:
                    xv = ts[:, b, sub_off : sub_off + ca * Lp].rearrange(
                        "p (s l) -> p s l", l=Lp
                    )
                    _tree_reduce(nc, xv, viol[:, vpos : vpos + ca], Lp)
                nc.vector.drain().then_inc(vsem, 1)
            # end tail: stats over all segments (after Pool relu/compare)
            vector.wait_ge(psem, nch)
            nc.vector.tensor_reduce(
                out=tout[:, 0:1], in_=viol[:],
                axis=mybir.AxisListType.X, op=mybir.AluOpType.add,
            )
            nc.vector.tensor_reduce(
                out=tout[:, 1:2], in_=viol[:],
                axis=mybir.AxisListType.X, op=mybir.AluOpType.max,
            )
            nc.vector.tensor_reduce(
                out=tout[:, 2:3], in_=gtv[:],
                axis=mybir.AxisListType.X, op=mybir.AluOpType.add,
            )
            nc.vector.drain().then_inc(esem, 1)

        @block.gpsimd
        def _(gpsimd):
            def relu_gt(g):
                (soff, fc, subs) = chunks[g]
                v0 = subs[0][3]
                v1 = subs[-1][3] + subs[-1][1]
                gpsimd.wait_ge(vsem, g + 1)
                vs = viol[:, v0:v1]
                nc.gpsimd.tensor_scalar_max(out=vs, in0=vs, scalar1=0.0)
                nc.gpsimd.drain()
                nc.gpsimd.tensor_scalar(
                    out=gtv[:, v0:v1], in0=vs,
                    scalar1=1e-6, scalar2=None, op0=mybir.AluOpType.is_gt,
                )
                nc.gpsimd.drain().then_inc(psem, 1)

            for g, (soff, fc, subs) in enumerate(chunks):
                b = g % NRING
                pm = pms[g]
                gpsimd.wait_ge(dsems[b], 16 * (g // NRING + 1))
                nc.gpsimd.tensor_tensor(
                    out=ts[:, b, :pm], in0=ts[:, b, :pm],
                    in1=ts[:, b, fc : fc + pm], op=mybir.AluOpType.mult,
                )
                nc.gpsimd.drain().then_inc(pmsem, 1)
                if g >= 1:
                    relu_gt(g - 1)
            relu_gt(nch - 1)

    return nc


def _stride_of(deg):
    """Allowed segment strides (edge slots + 1 bias slot) per degree."""
    need = deg + 1
    lp = np.maximum(24, ((need + 3) // 4) * 4)       # 24,28,...,44
    lp = np.where(need > 44, ((need + 7) // 8) * 8, lp)   # 48,56
    lp = np.where(need > 56, ((need + 15) // 16) * 16, lp)  # 64,80,...
    return lp.astype(np.int64)


def _plan_chunks(tier_list):
    """Pack (Lp, sa) tiers into DMA chunks of subblocks.

    First and last chunks are ~EDGE_SLOTS to shorten pipeline fill/drain;
    middles ~CHUNK_SLOTS. Returns (chunks, spp, tier_map): tier_map[i] is a
    list of (rank_start, rank_end, chunk_soff, sub_off, fc_of_chunk) spans.
    """
    total = sum(Lp * sa for (Lp, sa) in tier_list)
    mid_n = max(1, -(-(total - 2 * EDGE_SLOTS) // CHUNK_SLOTS))
    mid = (total - 2 * EDGE_SLOTS) // mid_n
    budgets = [EDGE_SLOTS] + [mid] * mid_n + [2 * EDGE_SLOTS]

    # ascending stride: small-tier subblock spam lands in the early chunks
    # where the vector engine still has slack
    order = sorted(range(len(tier_list)), key=lambda i: tier_list[i][0])
    raw_chunks = [[]]
    fill = 0
    for i in order:
        Lp, sa = tier_list[i]
        r = 0
        while r < sa:
            budget = budgets[min(len(raw_chunks) - 1, len(budgets) - 1)]
            room = (budget - fill) // Lp
            if room == 0:
                raw_chunks.append([])
                fill = 0
                continue
            take = min(sa - r, room)
            raw_chunks[-1].append((i, r, take, Lp))
            fill += take * Lp
            r += take

    chunks = []
    tier_map = {i: [] for i in range(len(tier_list))}
    soff = vpos = 0
    for ch in raw_chunks:
        fc = sum(t * lp for (_, _, t, lp) in ch)
        subs = []
        sub_off = 0
        for (i, r, take, Lp) in ch:
            subs.append((sub_off, take, Lp, vpos))
            tier_map[i].append((r, r + take, soff, sub_off, fc))
            sub_off += take * Lp
            vpos += take
        chunks.append((soff, fc, subs))
        soff += 2 * fc
    return chunks, vpos, tier_map


def kernel(**inputs) -> tuple:
    global LAST_P1_ARGS, LAST_P2_ARGS
    prob_bin = np.asarray(inputs["prob_bin"], dtype=np.float32)
    logits = np.asarray(inputs["logits_int_small"], dtype=np.float32)
    offsets = np.asarray(inputs["int_small_offsets"], dtype=np.float32)
    pred_l = np.asarray(inputs["pred_int_large"], dtype=np.float32)
    feat = np.asarray(inputs["edge_features"], dtype=np.float32).reshape(-1)
    cfeat = np.asarray(inputs["constraint_features"], dtype=np.float32)
    vfeat = np.asarray(inputs["variable_features"], dtype=np.float32)
    idx_bin = np.asarray(inputs["idx_bin"], dtype=np.int64)
    idx_s = np.asarray(inputs["idx_int_small"], dtype=np.int64)
    idx_l = np.asarray(inputs["idx_int_large"], dtype=np.int64)
    var_types = np.asarray(inputs["var_types"], dtype=np.int64)
    ei = np.asarray(inputs["edge_indices"], dtype=np.int64)
    n_vars = int(inputs["n_vars"])

    n_con = cfeat.shape[0]
    ns, ccls = logits.shape
    bias = np.ascontiguousarray(cfeat[:, BIAS_COL])
    lp_vals = np.ascontiguousarray(vfeat[:, LP_SOL_COL])
    con = ei[0]
    var = ei[1]
    ne = con.shape[0]

    # ---------------- launch 1: expected values ----------------
    rows_pp = -(-ns // NBINS)
    rcs = [16, 48]
    rmid = -(-(rows_pp - 64) // 3)
    while sum(rcs) < rows_pp:
        rcs.append(min(rmid, rows_pp - sum(rcs)))
    rcmax = max(rcs)
    ns_pad = NBINS * rows_pp
    lg_pad = np.zeros((ns_pad, ccls), dtype=NP_BF16)
    lg_pad[:ns] = logits
    of_pad = np.zeros(ns_pad, dtype=NP_BF16)
    of_pad[:ns] = offsets
    lg_r = lg_pad.reshape(NCORES, P, rows_pp * ccls)
    of_r = of_pad.reshape(NCORES, P, rows_pp)
    stream1 = np.concatenate([lg_r, of_r], axis=2)
    clspat = np.ascontiguousarray(
        np.tile(np.arange(ccls, dtype=NP_BF16), rcmax)[None].repeat(P, 0)
    )

    LAST_P1_ARGS = (rcs, ccls)
    nc1 = _build_phase1(*LAST_P1_ARGS)
    in1 = [{"stream": stream1[c], "clspat": clspat} for c in range(NCORES)]
    res1 = run_bass_kernel_spmd(nc1, in1, list(range(NCORES)))
    expected = np.concatenate(
        [res1.results[c]["expected"].reshape(-1) for c in range(NCORES)]
    )[:ns].astype(np.float32)

    # ---------------- host: assemble x (indexed copies) ----------------
    xfull = np.zeros(n_vars, dtype=np.float32)
    xfull[idx_bin] = prob_bin[:, 0]
    xfull[idx_s] = expected
    xfull[idx_l] = pred_l[:, 0]
    xfull = np.where(var_types == 0, lp_vals, xfull)
    xfull_bf = xfull.astype(NP_BF16)
    feat_bf = feat.astype(NP_BF16)
    nbias_bf = (-bias).astype(NP_BF16)

    # ---------------- host index prep (layout only) ----------------
    deg = np.bincount(con, minlength=n_con)
    order = np.argsort(con, kind="stable")
    run_start = np.zeros(n_con + 1, dtype=np.int64)
    np.cumsum(deg, out=run_start[1:])
    off_in_run = np.arange(ne, dtype=np.int64) - run_start[con[order]]
    con_sorted = con[order]
    var_sorted = var[order]
    feat_sorted = feat_bf[order]

    # stride tier per constraint: smallest allowed stride >= deg+1 (bias
    # slot). Few tiers keep the per-subblock instruction count low; strides
    # mostly multiples of 4 where the degree mass is.
    Lp_of = _stride_of(deg)
    tier_Ls = np.unique(Lp_of)

    # per tier: bin/rank assignment
    bin_of = np.zeros(n_con, dtype=np.int64)
    rank_of = np.zeros(n_con, dtype=np.int64)
    tier_list = []
    for Lp in tier_Ls:
        cons = np.nonzero(Lp_of == Lp)[0]
        rank_order = cons[np.argsort(-deg[cons], kind="stable")]
        ar = np.arange(rank_order.size, dtype=np.int64)
        bin_of[rank_order] = ar % NBINS
        rank_of[rank_order] = ar // NBINS
        sa = max(int(-(-rank_order.size // NBINS)), 1)
        tier_list.append((int(Lp), sa))

    chunks, spp, tier_map = _plan_chunks(tier_list)
    ftot = chunks[-1][0] + 2 * chunks[-1][1]

    # ---------------- host: scatter the edge stream ----------------
    stream2 = np.zeros(NBINS * ftot, dtype=NP_BF16)
    e_Lp = Lp_of[con_sorted]
    for t, (Lp, sa) in enumerate(tier_list):
        spans = tier_map[t]
        rstarts = np.array([s[0] for s in spans], dtype=np.int64)
        base = np.array(
            [soff + sub_off for (_, _, soff, sub_off, _) in spans],
            dtype=np.int64,
        )
        fcs = np.array([s[4] for s in spans], dtype=np.int64)

        def locs(ranks, slot):
            si = np.searchsorted(rstarts, ranks, side="right") - 1
            loc = base[si] + (ranks - rstarts[si]) * Lp + slot
            return loc, fcs[si]

        sel = np.nonzero(e_Lp == Lp)[0]
        cs = con_sorted[sel]
        loc, fc_e = locs(rank_of[cs], off_in_run[sel])
        flat = bin_of[cs] * ftot + loc
        stream2[flat] = xfull_bf[var_sorted[sel]]
        stream2[flat + fc_e] = feat_sorted[sel]
        # bias slot per real segment: (1, -bias) at slot Lp-1
        cons = np.nonzero(Lp_of == Lp)[0]
        locb, fc_b = locs(rank_of[cons], Lp - 1)
        flatb = bin_of[cons] * ftot + locb
        stream2[flatb] = np.array(1.0, dtype=NP_BF16)
        stream2[flatb + fc_b] = nbias_bf[cons]
    stream2 = stream2.reshape(NCORES, P, ftot)

    # ---------------- launch 2: segment reduce + loss partials ----------------
    LAST_P2_ARGS = (chunks, spp)
    nc2 = _build_phase2(*LAST_P2_ARGS)
    in2 = [{"stream": stream2[c]} for c in range(NCORES)]
    res2 = run_bass_kernel_spmd(nc2, in2, list(range(NCORES)))

    parts = np.stack([res2.results[c]["partials"] for c in range(NCORES)])
    vsum = parts[:, :, 0].astype(np.float64).sum()
    vmax = np.float32(max(parts[:, :, 1].max(), 0.0))
    vcnt = np.int32(round(float(parts[:, :, 2].astype(np.float64).sum())))
    mean_viol = np.float32(vsum / n_con)
    penalty = np.float32(
        np.float32(LAMBDA_MEAN) * mean_viol + np.float32(LAMBDA_MAX) * vmax
    )
    return penalty, mean_viol, vmax, vcnt


# revision 36
# speedup vs baseline: 1.6545x; 1.0056x over previous
"""Trainium2 Bass kernel for ConstraintViolationLoss (GNN message passing).

Strategy (8 NeuronCores, SPMD, bf16 streams):
  - Host does index-only layout prep: sort edges by constraint, pad each
    constraint's edge list to a degree-tier stride L (granularity 4) plus one
    extra slot carrying (1, -bias), so the per-constraint segment-sum directly
    yields Ax - bias with no separate bias stream or subtract.
  - Launch 1 (softmax expected-value head): exp on the scalar engine; the
    class-weighted numerator and denominator reduced together as one bf16
    pairwise add-tree (4D view) on the vector engine; reciprocal + combine.
  - Host assembles the per-edge-slot gathered value stream (pure indexed
    copies of input values / launch-1 output; no arithmetic).
  - Launch 2: DMA chunks (all degree tiers coalesced, small first/last chunk
    to shorten pipeline ends, 4-deep ring buffer). Per chunk the vector
    engine does one flat bf16 multiply and per-tier in-place pairwise
    add-tree segment sums; the Pool engine applies relu + threshold compare;
    final sum/max/count reductions once at the end.
  - Host combines the 1024 per-partition partial rows into the 4 scalars.
"""

import sys

sys.path.insert(0, "/opt/trn_rl_repo")

import numpy as np

import concourse.bass as bass
import concourse.mybir as mybir
from concourse.bass_utils import run_bass_kernel_spmd

P = 128
NCORES = 8
NBINS = P * NCORES
LAMBDA_MEAN, LAMBDA_MAX = 1.0, 0.1
BIAS_COL = 1
LP_SOL_COL = 8
CHUNK_SLOTS = 3584        # stream slots / partition in the middle chunks
EDGE_SLOTS = 1024         # first and last chunk size (pipeline fill/drain)
NRING = 4                 # phase-2 ring buffers
POOL_FRAC = 0.42          # fraction of each chunk multiplied on Pool
F32 = mybir.dt.float32
BF16 = mybir.dt.bfloat16
NP_BF16 = mybir.dt.np(BF16)

# shapes of the most recent build, for test harness introspection
LAST_P1_ARGS = None
LAST_P2_ARGS = None


def _tree_reduce(nc, xv, out, width):
    """Sum the last axis of xv [..., width] into out [...].

    In-place pairwise halvings (bf16 tensor_tensor add, 2x DVE mode) while the
    width stays even, then one tensor_reduce for the remainder. Returns the
    final emitted instruction (the last reader of xv's buffer).
    """
    h = width
    while h % 2 == 0 and h > 2:
        nh = h // 2
        nc.vector.drain()
        nc.vector.tensor_tensor(
            out=xv[..., :nh], in0=xv[..., :nh], in1=xv[..., nh:h],
            op=mybir.AluOpType.add,
        )
        h = nh
    nc.vector.drain()
    with nc.allow_low_precision("bf16 partial sums, validated vs f64 oracle"):
        return nc.vector.tensor_reduce(
            out=out, in_=xv[..., :h],
            axis=mybir.AxisListType.X, op=mybir.AluOpType.add,
        )


def _build_phase1(rcs, ccls=16):
    """expected = softmax(logits) @ cls + offsets.

    rcs: rows/partition per compute chunk (first small for pipeline fill).
    Stream layout: [logits chunk 0 | ... | logits chunk n-1 | offsets(all)].
    """
    rows_pp = sum(rcs)
    rcmax = max(rcs)
    nch = len(rcs)
    lgmax = rcmax * ccls
    nc = bass.Bass()
    st = nc.declare_dram_parameter(
        "stream", [P, rows_pp * (ccls + 1)], BF16, isOutput=False
    )
    cp = nc.declare_dram_parameter("clspat", [P, lgmax], BF16, isOutput=False)
    ex = nc.declare_dram_parameter("expected", [P, rows_pp], BF16, isOutput=True)

    lgoff = [0]
    for rc in rcs:
        lgoff.append(lgoff[-1] + rc * ccls)
    roff = [0]
    for rc in rcs:
        roff.append(roff[-1] + rc)

    with (
        nc.sbuf_tensor([P, lgoff[-1] * 2], BF16) as wt,
        nc.sbuf_tensor([P, rows_pp], BF16) as toff,
        nc.sbuf_tensor([P, lgmax], BF16) as tcp,
        nc.sbuf_tensor([P, 2, rcmax], F32) as dn,
        nc.sbuf_tensor([P, rcmax], F32) as rden,
        nc.sbuf_tensor([P, rcmax], F32) as tmu,
        nc.sbuf_tensor([P, rows_pp], BF16) as tout,
        nc.Block() as block,
        nc.semaphore("cs1") as cs1,
        nc.semaphore("cs2") as cs2,
        nc.semaphore("cs3") as cs3,
        nc.semaphore("asem") as asem,
        nc.semaphore("vsem") as vsem,
        nc.semaphore("osem") as osem,
    ):
        dsems = [nc.alloc_semaphore(f"dsem{g}") for g in range(nch)]

        @block.sync
        def _(sync):
            sync.dma_start(
                out=wt[:, 2 * lgoff[0] : 2 * lgoff[0] + rcs[0] * ccls],
                in_=st[:, lgoff[0] : lgoff[1]],
            ).then_inc(dsems[0], 16)
            sync.dma_start(
                out=tcp[:, : rcs[0] * ccls], in_=cp[:, : rcs[0] * ccls]
            ).then_inc(cs1, 16)
            sync.dma_start(
                out=wt[:, 2 * lgoff[1] : 2 * lgoff[1] + rcs[1] * ccls],
                in_=st[:, lgoff[1] : lgoff[2]],
            ).then_inc(dsems[1], 16)
            sync.dma_start(
                out=toff[:], in_=st[:, lgoff[-1] :]
            ).then_inc(cs3, 16)
            sync.dma_start(
                out=tcp[:, rcs[0] * ccls :], in_=cp[:, rcs[0] * ccls :]
            ).then_inc(cs2, 16)
            for g in range(2, nch):
                sync.dma_start(
                    out=wt[:, 2 * lgoff[g] : 2 * lgoff[g] + rcs[g] * ccls],
                    in_=st[:, lgoff[g] : lgoff[g + 1]],
                ).then_inc(dsems[g], 16)
            for g in range(nch):
                sync.wait_ge(vsem, g + 1)
                sync.dma_start(
                    out=ex[:, roff[g] : roff[g + 1]],
                    in_=tout[:, roff[g] : roff[g + 1]],
                ).then_inc(osem, 16)
            sync.wait_ge(osem, 16 * nch)

        @block.scalar
        def _(scalar):
            for g in range(nch):
                lg = wt[:, 2 * lgoff[g] : 2 * lgoff[g] + rcs[g] * ccls]
                scalar.wait_ge(dsems[g], 16)
                nc.scalar.activation(
                    out=lg, in_=lg, func=mybir.ActivationFunctionType.Exp,
                ).then_inc(asem, 1)

        @block.vector
        def _(vector):
            vector.wait_ge(cs1, 16)
            vector.wait_ge(cs3, 16)
            for g, rc in enumerate(rcs):
                lgn = rc * ccls
                w2 = wt[:, 2 * lgoff[g] : 2 * lgoff[g] + 2 * lgn]
                if g == 1:
                    vector.wait_ge(cs2, 16)
                vector.wait_ge(asem, g + 1)
                # numerator products next to exp so one 4D tree reduces both
                nc.vector.tensor_tensor(
                    out=w2[:, lgn:], in0=w2[:, :lgn], in1=tcp[:, :lgn],
                    op=mybir.AluOpType.mult,
                )
                v4 = w2.rearrange("p (t r c) -> p t r c", t=2, c=ccls)
                _tree_reduce(nc, v4, dn[:, :, :rc], ccls)
                nc.vector.drain()
                nc.vector.reciprocal(out=rden[:, :rc], in_=dn[:, 0, :rc])
                nc.vector.drain()
                nc.vector.tensor_tensor(
                    out=tmu[:, :rc], in0=dn[:, 1, :rc], in1=rden[:, :rc],
                    op=mybir.AluOpType.mult,
                )
                nc.vector.drain()
                nc.vector.tensor_tensor(
                    out=tout[:, roff[g] : roff[g + 1]],
                    in0=tmu[:, :rc], in1=toff[:, roff[g] : roff[g + 1]],
                    op=mybir.AluOpType.add,
                )
                nc.vector.drain().then_inc(vsem, 1)

    return nc


def _build_phase2(chunks, spp):
    """Per-core segment reduce + loss partials, single bf16 stream.

    chunks: list of (soff, fc, subblocks); subblocks: (sub_off, ca, Lp, vpos).
    Chunk g stream block is [xg subs... | ft subs...] at soff, 2*fc wide.
    spp: total segments per partition.
    """
    nch = len(chunks)
    fcmax = max(c[1] for c in chunks)
    ftot = chunks[-1][0] + 2 * chunks[-1][1]
    nc = bass.Bass()
    st = nc.declare_dram_parameter("stream", [P, ftot], BF16, isOutput=False)
    out_p = nc.declare_dram_parameter("partials", [P, 3], F32, isOutput=True)

    # register -1e-6 as a const AP for the Sign activation's bias
    cbias = nc.alloc_sbuf_tensor("const-neg-thresh", [P, 1], F32)
    nc.gpsimd.memset(cbias.ap(), -1e-6)
    nc.all_engine_barrier()
    nc.const_aps.aps[(F32, -1e-6)] = cbias.ap()

    # Pool multiplies (and first-halves) a leading segment-aligned prefix of
    # each chunk; DVE trees its own region first so Pool is never waited hot.
    # pool_split[g]: per subblock, q = segments of it inside the Pool prefix.
    pool_split = []
    for (soff, fc, subs) in chunks:
        budget = int(fc * POOL_FRAC)
        qs = []
        for (sub_off, ca, Lp, vpos) in subs:
            q = min(ca, max(0, budget - sub_off) // Lp)
            qs.append(q)
        pool_split.append(qs)

    with (
        nc.sbuf_tensor([P, NRING, 2 * fcmax], BF16) as ts,
        nc.sbuf_tensor([P, spp], BF16) as viol,
        nc.sbuf_tensor([P, spp], BF16) as gtv,
        nc.sbuf_tensor([P, 3], F32) as tout,
        nc.Block() as block,
        nc.semaphore("vsem") as vsem,
        nc.semaphore("asem") as asem,
        nc.semaphore("esem") as esem,
        nc.semaphore("osem") as osem,
    ):
        dsems = [nc.alloc_semaphore(f"dsem{b}") for b in range(NRING)]
        pmsem = nc.alloc_semaphore("pmsem")

        @block.sync
        def _(sync):
            for g, (soff, fc, subs) in enumerate(chunks):
                if g >= NRING:
                    sync.wait_ge(vsem, g - NRING + 1)
                b = g % NRING
                sync.dma_start(
                    out=ts[:, b, : 2 * fc], in_=st[:, soff : soff + 2 * fc]
                ).then_inc(dsems[b], 16)
            sync.wait_ge(esem, 1)
            sync.wait_ge(asem, 2)
            sync.dma_start(out=out_p[:], in_=tout[:]).then_inc(osem, 16)
            sync.wait_ge(osem, 16)

        @block.vector
        def _(vector):
            for g, (soff, fc, subs) in enumerate(chunks):
                b = g % NRING
                qs = pool_split[g]
                pend = max(
                    (sub_off + q * Lp for (sub_off, _, Lp, _), q in
                     zip(subs, qs) if q > 0), default=0,
                )
                vector.wait_ge(dsems[b], 16 * (g // NRING + 1))
                if pend < fc:
                    nc.vector.tensor_tensor(
                        out=ts[:, b, pend:fc], in0=ts[:, b, pend:fc],
                        in1=ts[:, b, fc + pend : 2 * fc],
                        op=mybir.AluOpType.mult,
                    )
                # own-region trees first: Pool's region is never waited hot
                for (sub_off, ca, Lp, vpos), q in zip(subs, qs):
                    if q < ca:
                        xv = ts[:, b, sub_off + q * Lp : sub_off + ca * Lp]
                        xv = xv.rearrange("p (s l) -> p s l", l=Lp)
                        _tree_reduce(nc, xv, viol[:, vpos + q : vpos + ca], Lp)
                vector.wait_ge(pmsem, g + 1)
                for (sub_off, ca, Lp, vpos), q in zip(subs, qs):
                    if q > 0:
                        xv = ts[:, b, sub_off : sub_off + q * Lp].rearrange(
                            "p (s l) -> p s l", l=Lp
                        )
                        _tree_reduce(nc, xv, viol[:, vpos : vpos + q], Lp)
                nc.vector.drain().then_inc(vsem, 1)
            # raw max over the pre-relu sums; host applies the final relu
            nc.vector.tensor_reduce(
                out=tout[:, 1:2], in_=viol[:],
                axis=mybir.AxisListType.X, op=mybir.AluOpType.max,
            )
            nc.vector.drain().then_inc(esem, 1)

        @block.scalar
        def _(scalar):
            # warmup loads the "small" act table (relu+sign) during DMA fill
            nc.scalar.activation(
                out=gtv[:, 0:1], in_=cbias.ap(),
                func=mybir.ActivationFunctionType.Relu,
            )
            nc.scalar.drain()
            scalar.wait_ge(vsem, nch)
            nc.scalar.activation(
                out=gtv[:], in_=viol[:],
                func=mybir.ActivationFunctionType.Relu,
                accum_out=tout[:, 0:1],
            ).then_inc(asem, 1)
            nc.scalar.drain()
            # sign(viol_pre - 1e-6) = +1 iff violated; host decodes the count
            nc.scalar.activation(
                out=gtv[:], in_=viol[:],
                func=mybir.ActivationFunctionType.Sign,
                bias=-1e-6,
                accum_out=tout[:, 2:3],
            ).then_inc(asem, 1)

        @block.gpsimd
        def _(gpsimd):
            for g, (soff, fc, subs) in enumerate(chunks):
                b = g % NRING
                qs = pool_split[g]
                pend = max(
                    (sub_off + q * Lp for (sub_off, _, Lp, _), q in
                     zip(subs, qs) if q > 0), default=0,
                )
                gpsimd.wait_ge(dsems[b], 16 * (g // NRING + 1))
                if pend > 0:
                    nc.gpsimd.tensor_tensor(
                        out=ts[:, b, :pend], in0=ts[:, b, :pend],
                        in1=ts[:, b, fc : fc + pend],
                        op=mybir.AluOpType.mult,
                    )
                nc.gpsimd.drain().then_inc(pmsem, 1)

    return nc


def _stride_of(deg):
    """Allowed segment strides (edge slots + 1 bias slot) per degree."""
    need = deg + 1
    lp = np.maximum(24, ((need + 3) // 4) * 4)       # 24,28,...,44
    lp = np.where(need > 44, ((need + 7) // 8) * 8, lp)   # 48,56
    lp = np.where(need > 56, ((need + 15) // 16) * 16, lp)  # 64,80,...
    return lp.astype(np.int64)


def _plan_chunks(tier_list):
    """Pack (Lp, sa) tiers into DMA chunks of subblocks.

    First and last chunks are ~EDGE_SLOTS to shorten pipeline fill/drain;
    middles ~CHUNK_SLOTS. Returns (chunks, spp, tier_map): tier_map[i] is a
    list of (rank_start, rank_end, chunk_soff, sub_off, fc_of_chunk) spans.
    """
    total = sum(Lp * sa for (Lp, sa) in tier_list)
    mid_n = max(1, -(-(total - 2 * EDGE_SLOTS) // CHUNK_SLOTS))
    mid = (total - 2 * EDGE_SLOTS) // mid_n
    budgets = [EDGE_SLOTS] + [mid] * mid_n + [2 * EDGE_SLOTS]

    # ascending stride: small-tier subblock spam lands in the early chunks
    # where the vector engine still has slack
    order = sorted(range(len(tier_list)), key=lambda i: tier_list[i][0])
    raw_chunks = [[]]
    fill = 0
    for i in order:
        Lp, sa = tier_list[i]
        r = 0
        while r < sa:
            budget = budgets[min(len(raw_chunks) - 1, len(budgets) - 1)]
            room = (budget - fill) // Lp
            if room == 0:
                raw_chunks.append([])
                fill = 0
                continue
            take = min(sa - r, room)
            raw_chunks[-1].append((i, r, take, Lp))
            fill += take * Lp
            r += take

    chunks = []
    tier_map = {i: [] for i in range(len(tier_list))}
    soff = vpos = 0
    for ch in raw_chunks:
        fc = sum(t * lp for (_, _, t, lp) in ch)
        subs = []
        sub_off = 0
        for (i, r, take, Lp) in ch:
            subs.append((sub_off, take, Lp, vpos))
            tier_map[i].append((r, r + take, soff, sub_off, fc))
            sub_off += take * Lp
            vpos += take
        chunks.append((soff, fc, subs))
        soff += 2 * fc
    return chunks, vpos, tier_map


def kernel(**inputs) -> tuple:
    global LAST_P1_ARGS, LAST_P2_ARGS
    prob_bin = np.asarray(inputs["prob_bin"], dtype=np.float32)
    logits = np.asarray(inputs["logits_int_small"], dtype=np.float32)
    offsets = np.asarray(inputs["int_small_offsets"], dtype=np.float32)
    pred_l = np.asarray(inputs["pred_int_large"], dtype=np.float32)
    feat = np.asarray(inputs["edge_features"], dtype=np.float32).reshape(-1)
    cfeat = np.asarray(inputs["constraint_features"], dtype=np.float32)
    vfeat = np.asarray(inputs["variable_features"], dtype=np.float32)
    idx_bin = np.asarray(inputs["idx_bin"], dtype=np.int64)
    idx_s = np.asarray(inputs["idx_int_small"], dtype=np.int64)
    idx_l = np.asarray(inputs["idx_int_large"], dtype=np.int64)
    var_types = np.asarray(inputs["var_types"], dtype=np.int64)
    ei = np.asarray(inputs["edge_indices"], dtype=np.int64)
    n_vars = int(inputs["n_vars"])

    n_con = cfeat.shape[0]
    ns, ccls = logits.shape
    bias = np.ascontiguousarray(cfeat[:, BIAS_COL])
    lp_vals = np.ascontiguousarray(vfeat[:, LP_SOL_COL])
    con = ei[0]
    var = ei[1]
    ne = con.shape[0]

    # ---------------- launch 1: expected values ----------------
    rows_pp = -(-ns // NBINS)
    rcs = [16, 48]
    rmid = -(-(rows_pp - 64) // 3)
    while sum(rcs) < rows_pp:
        rcs.append(min(rmid, rows_pp - sum(rcs)))
    rcmax = max(rcs)
    ns_pad = NBINS * rows_pp
    lg_pad = np.zeros((ns_pad, ccls), dtype=NP_BF16)
    lg_pad[:ns] = logits
    of_pad = np.zeros(ns_pad, dtype=NP_BF16)
    of_pad[:ns] = offsets
    lg_r = lg_pad.reshape(NCORES, P, rows_pp * ccls)
    of_r = of_pad.reshape(NCORES, P, rows_pp)
    stream1 = np.concatenate([lg_r, of_r], axis=2)
    clspat = np.ascontiguousarray(
        np.tile(np.arange(ccls, dtype=NP_BF16), rcmax)[None].repeat(P, 0)
    )

    LAST_P1_ARGS = (rcs, ccls)
    nc1 = _build_phase1(*LAST_P1_ARGS)
    in1 = [{"stream": stream1[c], "clspat": clspat} for c in range(NCORES)]
    res1 = run_bass_kernel_spmd(nc1, in1, list(range(NCORES)))
    expected = np.concatenate(
        [res1.results[c]["expected"].reshape(-1) for c in range(NCORES)]
    )[:ns].astype(np.float32)

    # ---------------- host: assemble x (indexed copies) ----------------
    xfull = np.zeros(n_vars, dtype=np.float32)
    xfull[idx_bin] = prob_bin[:, 0]
    xfull[idx_s] = expected
    xfull[idx_l] = pred_l[:, 0]
    xfull = np.where(var_types == 0, lp_vals, xfull)
    xfull_bf = xfull.astype(NP_BF16)
    feat_bf = feat.astype(NP_BF16)
    nbias_bf = (-bias).astype(NP_BF16)

    # ---------------- host index prep (layout only) ----------------
    deg = np.bincount(con, minlength=n_con)
    order = np.argsort(con, kind="stable")
    run_start = np.zeros(n_con + 1, dtype=np.int64)
    np.cumsum(deg, out=run_start[1:])
    off_in_run = np.arange(ne, dtype=np.int64) - run_start[con[order]]
    con_sorted = con[order]
    var_sorted = var[order]
    feat_sorted = feat_bf[order]

    # stride tier per constraint: smallest allowed stride >= deg+1 (bias
    # slot). Few tiers keep the per-subblock instruction count low; strides
    # mostly multiples of 4 where the degree mass is.
    Lp_of = _stride_of(deg)
    tier_Ls = np.unique(Lp_of)

    # per tier: bin/rank assignment
    bin_of = np.zeros(n_con, dtype=np.int64)
    rank_of = np.zeros(n_con, dtype=np.int64)
    tier_list = []
    for Lp in tier_Ls:
        cons = np.nonzero(Lp_of == Lp)[0]
        rank_order = cons[np.argsort(-deg[cons], kind="stable")]
        ar = np.arange(rank_order.size, dtype=np.int64)
        bin_of[rank_order] = ar % NBINS
        rank_of[rank_order] = ar // NBINS
        sa = max(int(-(-rank_order.size // NBINS)), 1)
        tier_list.append((int(Lp), sa))

    chunks, spp, tier_map = _plan_chunks(tier_list)
    ftot = chunks[-1][0] + 2 * chunks[-1][1]

    # ---------------- host: scatter the edge stream ----------------
    stream2 = np.zeros(NBINS * ftot, dtype=NP_BF16)
    e_Lp = Lp_of[con_sorted]
    for t, (Lp, sa) in enumerate(tier_list):
        spans = tier_map[t]
        rstarts = np.array([s[0] for s in spans], dtype=np.int64)
        base = np.array(
            [soff + sub_off for (_, _, soff, sub_off, _) in spans],
            dtype=np.int64,
        )
        fcs = np.array([s[4] for s in spans], dtype=np.int64)

        def locs(ranks, slot):
            si = np.searchsorted(rstarts, ranks, side="right") - 1
            loc = base[si] + (ranks - rstarts[si]) * Lp + slot
            return loc, fcs[si]

        sel = np.nonzero(e_Lp == Lp)[0]
        cs = con_sorted[sel]
        loc, fc_e = locs(rank_of[cs], off_in_run[sel])
        flat = bin_of[cs] * ftot + loc
        stream2[flat] = xfull_bf[var_sorted[sel]]
        stream2[flat + fc_e] = feat_sorted[sel]
        # bias slot per real segment: (1, -bias) at slot Lp-1
        cons = np.nonzero(Lp_of == Lp)[0]
        locb, fc_b = locs(rank_of[cons], Lp - 1)
        flatb = bin_of[cons] * ftot + locb
        stream2[flatb] = np.array(1.0, dtype=NP_BF16)
        stream2[flatb + fc_b] = nbias_bf[cons]
    stream2 = stream2.reshape(NCORES, P, ftot)

    # ---------------- launch 2: segment reduce + loss partials ----------------
    LAST_P2_ARGS = (chunks, spp)
    nc2 = _build_phase2(*LAST_P2_ARGS)
    in2 = [{"stream": stream2[c]} for c in range(NCORES)]
    res2 = run_bass_kernel_spmd(nc2, in2, list(range(NCORES)))

    parts = np.stack([res2.results[c]["partials"] for c in range(NCORES)])
    vsum = parts[:, :, 0].astype(np.float64).sum()
    vmax = np.float32(max(parts[:, :, 1].max(), 0.0))
    # col 2 is sum(sign(viol - 1e-6)) = n_violated - n_not_violated
    sign_total = parts[:, :, 2].astype(np.float64).sum()
    vcnt = np.int32(round((sign_total + NBINS * spp) / 2))
    mean_viol = np.float32(vsum / n_con)
    penalty = np.float32(
        np.float32(LAMBDA_MEAN) * mean_viol + np.float32(LAMBDA_MAX) * vmax
    )
    return penalty, mean_viol, vmax, vcnt
